# revision 19
# baseline (speedup 1.0000x reference)
"""Trainium2 Bass kernel for nn_Block2x2DiagProduct (butterfly product).

Strategy:
  The 10 block-2x2-diag butterfly factors compose into a single dense
  1024x1024 matrix W (parameters are shared across blocks within each
  factor, and the product of all stages is dense). W is composed on the
  host in float64 (tiny: 10 einsums over a 1024x1024 identity), so the
  device kernel is a dense matmul  out = x @ Wt  (Wt = W^T).

  Sharding: pure data parallel — batch dim of x split across 8 cores,
  Wt replicated.

  Per-core kernel (batch shard 4096 x 1024, f32):
    - Wt (4 MiB) resident in SBUF, loaded once.
    - For each 128-row tile of x: DMA in, PE-transpose the 8 [128,128]
      feature chunks (matmul contracts along partitions, so x must have
      features on partitions), then 16 accumulating float32r matmuls
      (full-rate on PE, vs 1/4-rate plain fp32) into two [128,512] PSUM
      tiles, DMA'd straight from PSUM back to DRAM.
"""

import os
import sys

for _p in ("/opt/trn_rl_repo", "/root/.axon_site/_ro/trn_rl_repo"):
    if os.path.isdir(_p) and _p not in sys.path:
        sys.path.insert(0, _p)

import numpy as np

import concourse.bacc as bacc
import concourse.bass as bass
import concourse.mybir as mybir
from concourse.bass_utils import run_bass_kernel_spmd
from concourse.masks import make_identity
from concourse.tile import TileContext

SIZE = 1024
M = 10  # number of butterfly factors
N_CORES = 8
P = 128
KC = SIZE // P  # 8 contraction chunks
NH = SIZE // 512  # 2 output halves per row tile

# Results of the last device run (for the test harness).
last_exec_time_ns = None
last_mean_exec_time_ns = None

_nc_cache = {}


def _compose_wt(params):
    """Compose the 10 butterfly factors into Wt (1024x1024, f64) such that
    out = x @ Wt. Row k of Wt is the transform applied to basis vector e_k,
    i.e. Wt = reference(I_1024)."""
    wt = np.eye(SIZE, dtype=np.float64)
    for i in reversed(range(M)):
        s = SIZE >> i
        y = wt.reshape(SIZE, SIZE // s, 2, s // 2)
        wt = np.einsum(
            "ijk,bnjk->bnik", params[i].astype(np.float64), y
        ).reshape(SIZE, SIZE)
    return wt


def _build_nc(rows):
    f32 = mybir.dt.float32
    f32r = mybir.dt.float32r
    nb = rows // P

    # Bacc (not raw Bass): its finalize() pipeline splits multi-sem waits
    # into EventSemaphore instructions (HW allows 1 sync-wait per inst).
    nc = bacc.Bacc(None, target_bir_lowering=False)
    x_d = nc.dram_tensor("x", [rows, SIZE], f32, kind="ExternalInput")
    w_d = nc.dram_tensor("w", [SIZE, SIZE], f32, kind="ExternalInput")
    o_d = nc.dram_tensor("o", [rows, SIZE], f32, kind="ExternalOutput")

    with TileContext(nc) as tc:
        with (
            tc.tile_pool(name="const", bufs=1) as const_pool,
            tc.tile_pool(name="xin", bufs=4) as xpool,
            tc.tile_pool(name="xt", bufs=2 * KC) as xtpool,
            tc.tile_pool(name="osb", bufs=3) as opool,
            tc.tile_pool(name="tpsum", bufs=4, space="PSUM") as tpsum,
            tc.tile_pool(name="mpsum", bufs=4, space="PSUM") as mpsum,
        ):
            ident = const_pool.tile([P, P], f32)
            make_identity(nc, ident[:])
            # Dummy PE op consuming the identity: walrus allows only one
            # sync-wait on (transpose-)matmuls, and without this the first
            # real transpose would need two (identity-ready + x-DMA).
            pst0 = tpsum.tile([P, P], f32, name="pst_warm", tag="pst")
            nc.tensor.transpose(pst0[:], ident[:], ident[:])
            # Wt resident in SBUF: partition p, chunk c holds Wt[c*128+p, :]
            w_sb = const_pool.tile([P, KC, SIZE], f32)
            nc.sync.dma_start(
                out=w_sb[:], in_=w_d[:, :].rearrange("(c p) n -> p c n", p=P)
            )
            # FP32r matmul operands must be produced rounded-to-FP32r.
            w_sbr = const_pool.tile([P, KC, SIZE], f32r)
            nc.vector.tensor_copy(out=w_sbr[:], in_=w_sb[:])

            for bt in range(nb):
                x_sb = xpool.tile([P, SIZE], f32)
                # bufs=4 matches the 8-lane HWDGE round-robin (2 DMAs/iter),
                # so the slot-WAW predecessor IS the own-lane predecessor and
                # the load fits the DMA struct's 2-sync-wait limit.
                nc.sync.dma_start(
                    out=x_sb[:], in_=x_d[bt * P : (bt + 1) * P, :]
                )
                # Transpose x tile: 8 chunks of [128b, 128f] -> [128f, 128b]
                xts = []
                for k in range(KC):
                    pst = tpsum.tile([P, P], f32, tag="pst")
                    nc.tensor.transpose(
                        pst[:], x_sb[:, k * P : (k + 1) * P], ident[:]
                    )
                    xt_k = xtpool.tile([P, P], f32r, tag="xt", name=f"xt{k}")
                    nc.vector.tensor_copy(out=xt_k[:], in_=pst[:])
                    xts.append(xt_k)
                # out[b, :] = sum_k x[b, k] * Wt[k, :]
                psos = [
                    mpsum.tile([P, 512], f32, tag="mm_psum", name=f"pso{nh}")
                    for nh in range(NH)
                ]
                for k in range(KC):
                    for nh in range(NH):
                        nc.tensor.matmul(
                            psos[nh][:],
                            xts[k][:],
                            w_sbr[:, k, nh * 512 : (nh + 1) * 512],
                            start=(k == 0),
                            stop=(k == KC - 1),
                        )
                o_sb = opool.tile([P, SIZE], f32)
                for nh in range(NH):
                    nc.vector.tensor_copy(
                        out=o_sb[:, nh * 512 : (nh + 1) * 512], in_=psos[nh][:]
                    )
                nc.sync.dma_start(
                    out=o_d[bt * P : (bt + 1) * P, :], in_=o_sb[:]
                )
    nc.finalize()
    return nc


def kernel(**inputs):
    global last_exec_time_ns, last_mean_exec_time_ns

    x = np.ascontiguousarray(np.asarray(inputs["x"], dtype=np.float32))
    params = [np.asarray(inputs[f"ABCD{i}"]) for i in range(M)]
    wt = np.ascontiguousarray(_compose_wt(params).astype(np.float32))

    batch = x.shape[0]
    assert batch % N_CORES == 0
    rows = batch // N_CORES

    if rows not in _nc_cache:
        _nc_cache[rows] = _build_nc(rows)
    nc = _nc_cache[rows]

    in_maps = [
        {"x": x[i * rows : (i + 1) * rows], "w": wt} for i in range(N_CORES)
    ]
    res = run_bass_kernel_spmd(nc, in_maps, core_ids=list(range(N_CORES)))
    last_exec_time_ns = res.exec_time_ns
    last_mean_exec_time_ns = res.mean_exec_time_ns

    return np.concatenate([r["o"] for r in res.results], axis=0)


# revision 20
# speedup vs baseline: 1.1819x; 1.1819x over previous
"""Trainium2 Bass kernel for nn_Block2x2DiagProduct (butterfly product).

Strategy:
  The 10 block-2x2-diag butterfly factors compose into a single dense
  1024x1024 matrix W (parameters are shared across blocks within each
  factor, so the product of all stages is dense). W is composed on the
  host in float64 (10 einsums over a 1024x1024 identity), so the device
  kernel is a dense matmul  out = x @ Wt  (Wt = W^T).

  Sharding: pure data parallel — batch dim of x split across 8 cores,
  Wt replicated.

  Per-core kernel (batch shard 4096 x 1024, f32):
    - Wt (4 MiB) resident in SBUF, loaded once (SWDGE, chunked, so the
      x loads on HWDGE aren't serialized behind it).
    - For each 128-row tile of x: DMA in, PE-transpose the 8 [128,128]
      feature chunks 4-up into [128,512] PSUM tiles (matmul contracts
      along partitions, so x needs features on partitions), single DVE
      cast per PSUM tile into float32r SBUF, then 16 accumulating
      float32r matmuls (full-rate on PE, vs 1/4-rate plain fp32) into
      two [128,512] PSUM tiles, copied to SBUF on the Scalar engine and
      DMA'd out.
"""

import os
import sys

for _p in ("/opt/trn_rl_repo", "/root/.axon_site/_ro/trn_rl_repo"):
    if os.path.isdir(_p) and _p not in sys.path:
        sys.path.insert(0, _p)

import numpy as np

import concourse.bacc as bacc
import concourse.bass as bass
import concourse.mybir as mybir
from concourse.bass_utils import run_bass_kernel_spmd
from concourse.masks import make_identity
from concourse.tile import TileContext

SIZE = 1024
M = 10  # number of butterfly factors
N_CORES = 8
P = 128
KC = SIZE // P  # 8 contraction chunks
NH = SIZE // 512  # 2 output halves per row tile

# Results of the last device run (for the test harness).
last_exec_time_ns = None
last_mean_exec_time_ns = None

_nc_cache = {}


def _compose_wt(params):
    """Compose the 10 butterfly factors into Wt (1024x1024, f64) such that
    out = x @ Wt. Row k of Wt is the transform applied to basis vector e_k,
    i.e. Wt = reference(I_1024)."""
    wt = np.eye(SIZE, dtype=np.float64)
    for i in reversed(range(M)):
        s = SIZE >> i
        y = wt.reshape(SIZE, SIZE // s, 2, s // 2)
        wt = np.einsum(
            "ijk,bnjk->bnik", params[i].astype(np.float64), y
        ).reshape(SIZE, SIZE)
    return wt


def _build_nc(rows):
    f32 = mybir.dt.float32
    f32r = mybir.dt.float32r
    nb = rows // P

    # Bacc (not raw Bass): its finalize() pipeline splits multi-sem waits
    # into EventSemaphore instructions (HW allows 1 sync-wait per inst).
    nc = bacc.Bacc(None, target_bir_lowering=False)
    x_d = nc.dram_tensor("x", [rows, SIZE], f32, kind="ExternalInput")
    w_d = nc.dram_tensor("w", [SIZE, SIZE], f32, kind="ExternalInput")
    o_d = nc.dram_tensor("o", [rows, SIZE], f32, kind="ExternalOutput")

    with TileContext(nc) as tc:
        with (
            tc.tile_pool(name="const", bufs=1) as const_pool,
            tc.tile_pool(name="xin", bufs=4) as xpool,
            tc.tile_pool(name="xt", bufs=4) as xtpool,
            tc.tile_pool(name="osb", bufs=3) as opool,
            tc.tile_pool(name="tpsum", bufs=4, space="PSUM") as tpsum,
            tc.tile_pool(name="mpsum", bufs=4, space="PSUM") as mpsum,
        ):
            ident = const_pool.tile([P, P], f32)
            make_identity(nc, ident[:])
            # Dummy PE op consuming the identity: walrus allows only one
            # sync-wait on (transpose-)matmuls, and without this the first
            # real transpose would need two (identity-ready + x-DMA).
            pst0 = tpsum.tile([P, P], f32, name="pst_warm", tag="pst")
            nc.tensor.transpose(pst0[:], ident[:], ident[:])

            # Wt resident in SBUF: partition p, chunk c holds Wt[c*128+p, :].
            # SWDGE + per-chunk loads: doesn't serialize the HWDGE x loads,
            # and chunk 0's float32r cast is ready early.
            w_sb = const_pool.tile([P, KC, SIZE], f32)
            w_sbr = const_pool.tile([P, KC, SIZE], f32r)
            for k in range(KC):
                nc.gpsimd.dma_start(
                    out=w_sb[:, k, :], in_=w_d[k * P : (k + 1) * P, :]
                )
                # FP32r matmul operands must be produced rounded-to-FP32r.
                nc.vector.tensor_copy(out=w_sbr[:, k, :], in_=w_sb[:, k, :])

            for bt in range(nb):
                x_sb = xpool.tile([P, SIZE], f32)
                # bufs=4 matches the 8-lane HWDGE round-robin (2 DMAs/iter),
                # so the slot-WAW predecessor IS the own-lane predecessor and
                # the load fits the DMA struct's sync-wait limit.
                nc.sync.dma_start(
                    out=x_sb[:], in_=x_d[bt * P : (bt + 1) * P, :]
                )
                # Transpose 8 chunks of [128b, 128f] -> [128f, 128b],
                # 4 chunks per PSUM bank, one cast per bank.
                xts = []
                for h in range(2):
                    pst = tpsum.tile([P, 512], f32, tag="pst", name=f"pst{h}")
                    for j in range(4):
                        k = 4 * h + j
                        nc.tensor.transpose(
                            pst[:, j * P : (j + 1) * P],
                            x_sb[:, k * P : (k + 1) * P],
                            ident[:],
                        )
                    xt_h = xtpool.tile([P, 512], f32r, tag="xt", name=f"xt{h}")
                    nc.vector.tensor_copy(out=xt_h[:], in_=pst[:])
                    xts.append(xt_h)
                # out[b, :] = sum_k x[b, k] * Wt[k, :]
                psos = [
                    mpsum.tile([P, 512], f32, tag="mm_psum", name=f"pso{nh}")
                    for nh in range(NH)
                ]
                for k in range(KC):
                    for nh in range(NH):
                        nc.tensor.matmul(
                            psos[nh][:],
                            xts[k // 4][:, (k % 4) * P : (k % 4 + 1) * P],
                            w_sbr[:, k, nh * 512 : (nh + 1) * 512],
                            start=(k == 0),
                            stop=(k == KC - 1),
                        )
                o_sb = opool.tile([P, SIZE], f32)
                for nh in range(NH):
                    nc.scalar.copy(
                        out=o_sb[:, nh * 512 : (nh + 1) * 512], in_=psos[nh][:]
                    )
                nc.sync.dma_start(
                    out=o_d[bt * P : (bt + 1) * P, :], in_=o_sb[:]
                )
    nc.finalize()
    return nc


def kernel(**inputs):
    global last_exec_time_ns, last_mean_exec_time_ns

    x = np.ascontiguousarray(np.asarray(inputs["x"], dtype=np.float32))
    params = [np.asarray(inputs[f"ABCD{i}"]) for i in range(M)]
    wt = np.ascontiguousarray(_compose_wt(params).astype(np.float32))

    batch = x.shape[0]
    assert batch % N_CORES == 0
    rows = batch // N_CORES

    if rows not in _nc_cache:
        _nc_cache[rows] = _build_nc(rows)
    nc = _nc_cache[rows]

    in_maps = [
        {"x": x[i * rows : (i + 1) * rows], "w": wt} for i in range(N_CORES)
    ]
    res = run_bass_kernel_spmd(nc, in_maps, core_ids=list(range(N_CORES)))
    last_exec_time_ns = res.exec_time_ns
    last_mean_exec_time_ns = res.mean_exec_time_ns

    return np.concatenate([r["o"] for r in res.results], axis=0)


# revision 21
# speedup vs baseline: 1.1841x; 1.0019x over previous
"""Trainium2 Bass kernel for nn_Block2x2DiagProduct (butterfly product).

Strategy:
  Stages 1..9 of the butterfly (all with block size <= 512) compose into
  blockdiag(R, R) with a single dense 512x512 matrix R shared by both
  halves (parameters are shared across blocks within each factor). The
  final stage (block size 1024) is a columnwise 2x2 butterfly:

      out[:, k]     = A[k]*y[:, k] + B[k]*y[:, 512+k]
      out[:, 512+k] = C[k]*y[:, k] + D[k]*y[:, 512+k]

  where y = x @ blockdiag(R^T, R^T). So the device kernel is two K=512
  float32r matmuls per row tile (PE) plus six columnwise multiply/adds
  (split across Vector and GpSimd, with Scalar doing the PSUM->SBUF
  staging). This halves the PE matmul work vs composing one dense
  1024x1024 matrix, moving the peeled stage to otherwise-idle engines.

  R is composed on the host in float64 (9 einsums over a 512x512
  identity). Sharding: pure data parallel — batch dim of x split across
  8 cores; R^T (1 MiB) and the stage-0 coefficients are replicated.

  Per-core per 128-row tile of x:
    - HWDGE DMA in; PE-transposes the 8 [128,128] feature chunks 4-up
      into [128,512] PSUM tiles (matmul contracts along partitions, so
      x needs features on partitions); Scalar-engine casts move them to
      SBUF as float32r (full-rate on PE, vs 1/4-rate plain fp32).
    - 8 accumulating float32r matmuls -> y_lo, y_hi in PSUM.
    - Butterfly: Vector computes A*y_lo + B*y_hi (reading PSUM), Scalar
      stages y_lo/y_hi to SBUF, GpSimd computes C*y_lo + D*y_hi (GpSimd
      cannot read PSUM), both into the output tile; HWDGE DMA out.
"""

import os
import sys

for _p in ("/opt/trn_rl_repo", "/root/.axon_site/_ro/trn_rl_repo"):
    if os.path.isdir(_p) and _p not in sys.path:
        sys.path.insert(0, _p)

import numpy as np

import concourse.bacc as bacc
import concourse.bass as bass
import concourse.mybir as mybir
from concourse.bass_utils import run_bass_kernel_spmd
from concourse.masks import make_identity
from concourse.tile import TileContext

SIZE = 1024
HALF = SIZE // 2
M = 10  # number of butterfly factors
N_CORES = 8
P = 128
KC = HALF // P  # 4 contraction chunks per half

# Results of the last device run (for the test harness).
last_exec_time_ns = None
last_mean_exec_time_ns = None

_nc_cache = {}


def _compose_w1t(params):
    """Compose butterfly stages 1..9 into W1t (512x512, f64) such that
    y_half = x_half @ W1t for each 512 half. Both halves share W1t because
    each factor's parameters are shared across its blocks."""
    w = np.eye(HALF, dtype=np.float64)
    for i in reversed(range(1, M)):
        s = SIZE >> i
        y = w.reshape(HALF, HALF // s, 2, s // 2)
        w = np.einsum(
            "ijk,bnjk->bnik", params[i].astype(np.float64), y
        ).reshape(HALF, HALF)
    return w


def _build_nc(rows):
    f32 = mybir.dt.float32
    f32r = mybir.dt.float32r
    nb = rows // P

    # Bacc (not raw Bass): its finalize() pipeline splits multi-sem waits
    # into EventSemaphore instructions (HW allows 1 sync-wait per inst).
    nc = bacc.Bacc(None, target_bir_lowering=False)
    x_d = nc.dram_tensor("x", [rows, SIZE], f32, kind="ExternalInput")
    w_d = nc.dram_tensor("w", [HALF, HALF], f32, kind="ExternalInput")
    coef_d = nc.dram_tensor("coef", [P, 4, HALF], f32, kind="ExternalInput")
    o_d = nc.dram_tensor("o", [rows, SIZE], f32, kind="ExternalOutput")

    with TileContext(nc) as tc:
        with (
            tc.tile_pool(name="const", bufs=1) as const_pool,
            tc.tile_pool(name="xin", bufs=4) as xpool,
            tc.tile_pool(name="xt", bufs=4) as xtpool,
            tc.tile_pool(name="stage", bufs=4) as spool,
            tc.tile_pool(name="osb", bufs=4) as opool,
            tc.tile_pool(name="tpsum", bufs=4, space="PSUM") as tpsum,
            tc.tile_pool(name="mpsum", bufs=4, space="PSUM") as mpsum,
        ):
            ident = const_pool.tile([P, P], f32)
            make_identity(nc, ident[:])
            # Dummy PE op consuming the identity: walrus allows only one
            # sync-wait on (transpose-)matmuls, and without this the first
            # real transpose would need two (identity-ready + x-DMA).
            pst0 = tpsum.tile([P, P], f32, name="pst_warm", tag="pst")
            nc.tensor.transpose(pst0[:], ident[:], ident[:])

            # W1t resident in SBUF: partition p, chunk c holds W1t[c*128+p, :].
            # SWDGE + per-chunk loads: doesn't serialize the HWDGE x loads,
            # and chunk 0's float32r cast is ready early.
            w_sb = const_pool.tile([P, KC, HALF], f32)
            w_sbr = const_pool.tile([P, KC, HALF], f32r)
            for c in range(KC):
                nc.gpsimd.dma_start(
                    out=w_sb[:, c, :], in_=w_d[c * P : (c + 1) * P, :]
                )
                # FP32r matmul operands must be produced rounded-to-FP32r.
                nc.vector.tensor_copy(out=w_sbr[:, c, :], in_=w_sb[:, c, :])
            # Stage-0 coefficients A,B,C,D, pre-replicated across partitions.
            coef_sb = const_pool.tile([P, 4, HALF], f32)
            nc.gpsimd.dma_start(out=coef_sb[:], in_=coef_d[:, :, :])
            cA = coef_sb[:, 0, :]
            cB = coef_sb[:, 1, :]
            cC = coef_sb[:, 2, :]
            cD = coef_sb[:, 3, :]

            for bt in range(nb):
                x_sb = xpool.tile([P, SIZE], f32)
                # bufs=4 matches the 8-lane HWDGE round-robin (2 DMAs/iter),
                # so the slot-WAW predecessor IS the own-lane predecessor and
                # the load fits the DMA struct's sync-wait limit.
                nc.sync.dma_start(
                    out=x_sb[:], in_=x_d[bt * P : (bt + 1) * P, :]
                )
                # Transpose 8 chunks of [128b, 128f] -> [128f, 128b],
                # 4 chunks per PSUM bank, one Scalar-engine cast per bank.
                xts = []
                for h in range(2):
                    pst = tpsum.tile([P, HALF], f32, tag="pst", name=f"pst{h}")
                    for j in range(KC):
                        k = KC * h + j
                        nc.tensor.transpose(
                            pst[:, j * P : (j + 1) * P],
                            x_sb[:, k * P : (k + 1) * P],
                            ident[:],
                        )
                    xt_h = xtpool.tile([P, HALF], f32r, tag="xt", name=f"xt{h}")
                    nc.scalar.copy(out=xt_h[:], in_=pst[:])
                    xts.append(xt_h)
                # y_half[b, :] = sum_k x_half[b, k] * W1t[k, :]
                psos = [
                    mpsum.tile([P, HALF], f32, tag="mm_psum", name=f"pso{h}")
                    for h in range(2)
                ]
                for c in range(KC):
                    for h in range(2):
                        nc.tensor.matmul(
                            psos[h][:],
                            xts[h][:, c * P : (c + 1) * P],
                            w_sbr[:, c, :],
                            start=(c == 0),
                            stop=(c == KC - 1),
                        )
                # Peeled stage 0: out_lo = A*y_lo + B*y_hi (Vector, reads
                # PSUM); out_hi = C*y_lo + D*y_hi (GpSimd, from the Scalar
                # engine's SBUF staging copies).
                o_sb = opool.tile([P, SIZE], f32)
                c_lo = spool.tile([P, HALF], f32, tag="c_lo", name="c_lo")
                c_hi = spool.tile([P, HALF], f32, tag="c_hi", name="c_hi")
                nc.scalar.copy(out=c_lo[:], in_=psos[0][:])
                nc.scalar.copy(out=c_hi[:], in_=psos[1][:])
                t0 = spool.tile([P, HALF], f32, tag="t0", name="t0")
                t1 = spool.tile([P, HALF], f32, tag="t1", name="t1")
                nc.vector.tensor_mul(t0[:], psos[0][:], cA)
                nc.vector.tensor_mul(t1[:], psos[1][:], cB)
                nc.vector.tensor_add(o_sb[:, :HALF], t0[:], t1[:])
                t2 = spool.tile([P, HALF], f32, tag="t2", name="t2")
                t3 = spool.tile([P, HALF], f32, tag="t3", name="t3")
                nc.gpsimd.tensor_mul(t2[:], c_lo[:], cC)
                nc.gpsimd.tensor_mul(t3[:], c_hi[:], cD)
                nc.gpsimd.tensor_add(o_sb[:, HALF:], t2[:], t3[:])
                nc.sync.dma_start(
                    out=o_d[bt * P : (bt + 1) * P, :], in_=o_sb[:]
                )
    nc.finalize()
    return nc


def kernel(**inputs):
    global last_exec_time_ns, last_mean_exec_time_ns

    x = np.ascontiguousarray(np.asarray(inputs["x"], dtype=np.float32))
    params = [np.asarray(inputs[f"ABCD{i}"]) for i in range(M)]
    w1t = np.ascontiguousarray(_compose_w1t(params).astype(np.float32))
    abcd0 = params[0].astype(np.float32)  # (2, 2, 512)
    coef = np.ascontiguousarray(
        np.broadcast_to(
            abcd0.reshape(1, 4, HALF), (P, 4, HALF)
        ).astype(np.float32)
    )

    batch = x.shape[0]
    assert batch % N_CORES == 0
    rows = batch // N_CORES

    if rows not in _nc_cache:
        _nc_cache[rows] = _build_nc(rows)
    nc = _nc_cache[rows]

    in_maps = [
        {"x": x[i * rows : (i + 1) * rows], "w": w1t, "coef": coef}
        for i in range(N_CORES)
    ]
    res = run_bass_kernel_spmd(nc, in_maps, core_ids=list(range(N_CORES)))
    last_exec_time_ns = res.exec_time_ns
    last_mean_exec_time_ns = res.mean_exec_time_ns

    return np.concatenate([r["o"] for r in res.results], axis=0)
